# revision 31
# baseline (speedup 1.0000x reference)
"""Trainium2 Bass kernel: fused multi-head attention block (projections +
softmax attention + output projection + residual + LayerNorm).

Sharding: 8 cores = 2 batches x 4 token-chunks of 512. Each core projects
K/V/Q for its OWN 512 tokens; V (fp8, with fused ones column) and the
K^T tail (d'-tiles NLOC..7) are AllGather-ed across the 4-core batch
group. K^T for d'-tiles 0..NLOC-1 is recomputed locally over the full
sequence so attention starts while the gathers are in flight. Each core
then runs attention for its 512 queries over all 2048 keys, the output
projection, residual add and LayerNorm.

P*V runs in fp8e4m3 with DoubleRow matmuls (2 key-tiles per pass); exp
is shifted by -7 so softmax numerators fit fp8 range (denominator
scales identically, so the quotient is unchanged).

Device-side layouts (per core):
  xt   [1024, 2048] bf16  x[b]^T, natural token order (local K path)
  xo   [1024, 512]  bf16  x[b]^T own-token columns (projections)
  xq   [512, 1024]  f32   own-token rows of x[b] (residual input)
  wq/wk/wv [1024, 1024] bf16  [c, h*64+d] (head-minor)
  wo   [1024, 1024] bf16  [(h*64+d), m]
  bias [16, 128]    f32   additive key bias per key tile/partition
                          (mask bias - EXP_SHIFT)
  gamma/beta [1024] bf16
Output: y [512, 1024] f32.
"""

import contextlib

import numpy as np
import ml_dtypes

import concourse.bass as bass
import concourse.tile as tile
from concourse import mybir
from concourse import bass_utils

BF16 = ml_dtypes.bfloat16
N_CORES = 8
B, L, D, H, DH = 2, 2048, 1024, 16, 64
Q = L // 4          # tokens owned per core
CT = D // 128       # contraction tiles over features
JT = L // 128       # key tiles
IT = Q // 128       # query tiles per core
LN_EPS = 1e-5
GROUPS = [[0, 1, 2, 3], [4, 5, 6, 7]]
NLOC = 2            # d'-tiles whose K^T is recomputed locally

EXP_SHIFT = 0.0

F32 = mybir.dt.float32
BF = mybir.dt.bfloat16
E8 = mybir.dt.float8e4
PVDT = BF           # softmax numerators (P) stay bf16
VDT = E8            # V is gathered/stored in fp8e4m3 (stationary operand)
FV = H * (DH + 1)   # V row width incl ones columns
FVH = 8 * (DH + 1)  # half of it (8 heads)


def _split_waits(nc, maxw=1):
    """This walrus build rejects instructions with more than one sync wait;
    split excess waits into preceding NOPs on the same engine."""
    ctr = 0
    for fn in nc.m.functions:
        for bb in fn.blocks:
            new_insts = []
            for inst in bb.instructions:
                si = inst.sync_info
                if si is not None and len(si.on_wait) > maxw:
                    waits = list(si.on_wait)
                    excess, keep = waits[:-maxw], waits[-maxw:]
                    for i in range(0, len(excess), maxw):
                        ctr += 1
                        new_insts.append(mybir.InstNoOp(
                            name=f"waitsplit_nop_{ctr}",
                            engine=inst.engine,
                            sync_info=mybir.SyncInfo(
                                on_wait=excess[i:i + maxw], on_update=[]),
                            text_hint="waitsplit",
                        ))
                    si.on_wait = keep
                new_insts.append(inst)
            bb.instructions = new_insts
    return ctr


def _emit(nc, tc, hh, masked, plain_ln):
    Exp = mybir.ActivationFunctionType.Exp
    Sqrt = mybir.ActivationFunctionType.Sqrt
    DR = mybir.MatmulPerfMode.DoubleRow

    xt_ap = hh["xt"].ap().rearrange("(t p) l -> p t l", p=128)   # [128,8,2048]
    xo_ap = hh["xo"].ap().rearrange("(t p) q -> p t q", p=128)   # [128,8,512]
    wq_ap = hh["wq"].ap().rearrange("(t p) d -> p t d", p=128)
    wk_ap = hh["wk"].ap().rearrange("(t p) d -> p t d", p=128)
    wv_ap = hh["wv"].ap().rearrange("(t p) d -> p t d", p=128)
    wo_ap = hh["wo"].ap().rearrange("(t p) d -> p t d", p=128)
    bias_ap = hh["bias"].ap().rearrange("a b -> b a")            # [128,16]
    xq_ap = hh["xq"].ap().rearrange("(t p) d -> p t d", p=128)   # [128,4,1024]
    y_ap = hh["y"].ap()

    def bcast_dram(h1d, parts=128):
        a = h1d.ap()
        return bass.AP(tensor=a.tensor, offset=a.offset,
                       ap=[[0, parts]] + list(a.ap))

    with contextlib.ExitStack() as ctx:
        dram = ctx.enter_context(tc.tile_pool(name="dram", bufs=1,
                                              space="DRAM"))
        const = ctx.enter_context(tc.tile_pool(name="const", bufs=1))
        xtp = ctx.enter_context(tc.tile_pool(name="xtp", bufs=2))
        wpool = ctx.enter_context(tc.tile_pool(name="wpool", bufs=2))
        wkp = ctx.enter_context(tc.tile_pool(name="wkp", bufs=1))
        vctp = ctx.enter_context(tc.tile_pool(name="vctp", bufs=1))
        qtp = ctx.enter_context(tc.tile_pool(name="qtp", bufs=1))
        ktp = ctx.enter_context(tc.tile_pool(name="ktp", bufs=4))
        vp = ctx.enter_context(tc.tile_pool(name="vp", bufs=2))
        expp = ctx.enter_context(tc.tile_pool(name="expp", bufs=2))
        ptp = ctx.enter_context(tc.tile_pool(name="ptp", bufs=1))
        npool = ctx.enter_context(tc.tile_pool(name="npool", bufs=2))
        xqp = ctx.enter_context(tc.tile_pool(name="xqp", bufs=2))
        lnp = ctx.enter_context(tc.tile_pool(name="lnp", bufs=2))
        statp = ctx.enter_context(tc.tile_pool(name="statp", bufs=4))
        psS = ctx.enter_context(tc.tile_pool(name="psS", bufs=2, space="PSUM"))
        psP = ctx.enter_context(tc.tile_pool(name="psP", bufs=4, space="PSUM"))

        # DRAM bounce buffers; gather order is V(heads 0-7), K tail,
        # V(heads 8-15) so each lands just before its first consumer
        vc1 = dram.tile([Q, FVH], VDT)
        vg1 = dram.tile([4, Q, FVH], VDT)
        vc2 = dram.tile([Q, FVH], VDT)
        vg2 = dram.tile([4, Q, FVH], VDT)
        KD = 8 - NLOC
        kc = dram.tile([KD * 128, Q], BF)
        kg = dram.tile([4, KD * 128, Q], BF)

        # ---- constants / small loads ----
        eps_sb = const.tile([128, 1], F32)
        nc.vector.memset(eps_sb[:], LN_EPS)
        shift_sb = const.tile([128, 1], F32)
        nc.vector.memset(shift_sb[:], -EXP_SHIFT)
        ones64 = const.tile([65, 64], F32)
        nc.vector.memset(ones64[:], 1.0)
        bias_sb = const.tile([128, 16], F32)
        den_st = const.tile([65, 512], F32)
        nc.vector.memset(den_st[:], 1.0)
        rdiv_st = const.tile([65, 512], F32)
        gamma_sb = const.tile([128, 1024], BF)
        beta_sb = const.tile([128, 1024], BF)

        # ---- input streams: xo/xt on SP queue, weights on ACT queue ----
        xo_sb = const.tile([128, CT, Q], BF)
        nc.sync.dma_start(out=xo_sb[:], in_=xo_ap)
        wk_sb = wkp.tile([128, CT, 1024], BF)
        nc.sync.dma_start(out=wk_sb[:], in_=wk_ap)
        xt_pre = []
        for c in range(2):
            t = xtp.tile([128, CT, 512], BF, tag="xt", name=f"xtpre{c}")
            nc.gpsimd.dma_start(out=t[:],
                                in_=xt_ap[:, :, c * 512:(c + 1) * 512])
            xt_pre.append(t)
        warm = const.tile([1, 1], F32)
        nc.scalar.activation(warm[:], eps_sb[0:1, 0:1], Exp,
                             bias=0.0, scale=1.0)
        wv_sb = wpool.tile([128, CT, 1024], BF, tag="w")
        nc.scalar.dma_start(out=wv_sb[:, 0:4, :], in_=wv_ap[:, 0:4, :])
        nc.scalar.dma_start(out=wv_sb[:, 4:8, :], in_=wv_ap[:, 4:8, :])
        wq_sb = wpool.tile([128, CT, 1024], BF, tag="w")
        nc.scalar.dma_start(out=wq_sb[:], in_=wq_ap)

        # ---- V projection (own tokens): [token 128][h*64] + ones col ----
        vct = vctp.tile([128, 4, H, DH + 1], VDT)
        nc.vector.memset(vct[:, :, :, DH:DH + 1], 1.0)
        for lt in range(4):
            ps = psS.tile([128, 2, 512], F32, tag="ss")
            for nt in range(2):
                for ct in range(CT):
                    nc.tensor.matmul(
                        ps[:, nt, :],
                        xo_sb[:, ct, lt * 128:(lt + 1) * 128],
                        wv_sb[:, ct, nt * 512:(nt + 1) * 512],
                        start=(ct == 0), stop=(ct == CT - 1))
            nc.vector.tensor_copy(
                vct[:, lt, :, 0:DH],
                ps.rearrange("p n (h d) -> p (n h) d", h=8))
        nc.gpsimd.dma_start(
            out=vc1[:].rearrange("(t p) f -> p t f", p=128),
            in_=vct[:, :, 0:8, :].rearrange("p t h e -> p t (h e)"))
        nc.gpsimd.collective_compute(
            "AllGather", mybir.AluOpType.bypass, replica_groups=GROUPS,
            ins=[vc1[:].opt()], outs=[vg1[:].opt()])

        # ---- Q^T projection part 1 (d'-tiles 0/1) so attention can
        # start; local K^T for d'-tile 0 runs before the own-key pass ----
        probt = ptp.tile([128, 8, Q], BF)
        kct = probt
        qt_all = qtp.tile([128, 8, Q], BF)

        def q_proj(dtp):
            ps = psS.tile([128, 2, 512], F32, tag="ss")
            for half in range(2):
                dt = 2 * dtp + half
                for ct in range(CT):
                    nc.tensor.matmul(
                        ps[:, half, :],
                        wq_sb[:, ct, dt * 128:(dt + 1) * 128],
                        xo_sb[:, ct, :],
                        start=(ct == 0), stop=(ct == CT - 1))
            nc.vector.tensor_copy(qt_all[:, 2 * dtp:2 * dtp + 2, :], ps[:])

        # ---- K^T projection (own keys, d'-tiles NLOC..7 only) ----
        # staged in the probt tile (dead until phase B; kc DMA orders reuse)
        def k_own():
            for i, dt in enumerate(range(NLOC, 8)):
                half = i % 2
                if half == 0:
                    ps = psS.tile([128, 2, 512], F32, tag="ss")
                for ct in range(CT):
                    nc.tensor.matmul(
                        ps[:, half, :],
                        wk_sb[:, ct, dt * 128:(dt + 1) * 128],
                        xo_sb[:, ct, :],
                        start=(ct == 0), stop=(ct == CT - 1))
                if half == 1 or i == KD - 1:
                    nc.vector.tensor_copy(
                        kct[:, i - half:i + 1, :], ps[:, 0:half + 1, :])
            nc.gpsimd.dma_start(
                out=kc[:].rearrange("(t p) q -> p t q", p=128),
                in_=kct[:, 0:KD, :])
            nc.gpsimd.collective_compute(
                "AllGather", mybir.AluOpType.bypass, replica_groups=GROUPS,
                ins=[kc[:].opt()], outs=[kg[:].opt()])
            nc.gpsimd.dma_start(
                out=vc2[:].rearrange("(t p) f -> p t f", p=128),
                in_=vct[:, :, 8:16, :].rearrange("p t h e -> p t (h e)"))
            nc.gpsimd.collective_compute(
                "AllGather", mybir.AluOpType.bypass, replica_groups=GROUPS,
                ins=[vc2[:].opt()], outs=[vg2[:].opt()])

        # remaining loads for phases B/C
        wo_sb = wpool.tile([128, CT, 1024], BF, tag="w")
        nc.scalar.dma_start(out=wo_sb[:], in_=wo_ap)
        nc.scalar.dma_start(out=bias_sb[:], in_=bias_ap)
        nc.scalar.dma_start(out=gamma_sb[:], in_=bcast_dram(hh["gamma"]))
        nc.scalar.dma_start(out=beta_sb[:], in_=bcast_dram(hh["beta"]))

        # ---- attention loop over d'-tiles (= head pairs) ----
        def local_k(dt, kt_t, pre=None, cps=(0, 1)):
            # local K^T over the full (natural-order) sequence,
            # x^T streamed chunk-by-chunk from DRAM
            for cp in cps:
                ps = psS.tile([128, 2, 512], F32, tag="ss")
                for hf in range(2):
                    c = 2 * cp + hf
                    if pre is not None and c < len(pre):
                        xt_c = pre[c]
                    else:
                        xt_c = xtp.tile([128, CT, 512], BF, tag="xt")
                        nc.sync.dma_start(
                            out=xt_c[:],
                            in_=xt_ap[:, :, c * 512:(c + 1) * 512])
                    for ct in range(CT):
                        nc.tensor.matmul(
                            ps[:, hf, :],
                            wk_sb[:, ct, dt * 128:(dt + 1) * 128],
                            xt_c[:, ct, :],
                            start=(ct == 0), stop=(ct == CT - 1))
                nc.vector.tensor_copy(
                    kt_t[:, 2 * cp:2 * cp + 2, :], ps[:])

        def fetch_k(dt, kt_t):
            nc.sync.dma_start(
                out=kt_t[:],
                in_=kg[:, (dt - NLOC) * 128:(dt - NLOC + 1) * 128, :]
                .rearrange("c p q -> p c q"))

        def normalize(pend):
            pdt, pv_sbs = pend
            nc.vector.reciprocal(rdiv_st[:], den_st[:])
            for hb in range(2):
                ps_b = psP.tile([64, 512], F32, tag="pp", name=f"bb{hb}")
                nc.tensor.matmul(ps_b[:],
                                 ones64[hb * 64:hb * 64 + 1, :],
                                 rdiv_st[hb * 64:hb * 64 + 1, :],
                                 start=True, stop=True)
                nc.vector.tensor_mul(
                    probt[hb * 64:hb * 64 + 64, pdt, :],
                    pv_sbs[hb][:], ps_b[:])

        kts = {0: ktp.tile([128, 4, Q], BF, tag="kt", name="kt0"),
               1: ktp.tile([128, 4, Q], BF, tag="kt", name="kt1")}
        k_own()
        local_k(0, kts[0], pre=xt_pre)
        q_proj(0)
        pending = None          # (pdt, pv_sbs) awaiting normalize
        prevpv = None           # (pdt, pv_list, v_t, expt, base_jt)

        def emit_pv(pp, jj):
            pdt, pvl, v_p, e_p, bjt = pp
            jt = bjt + jj
            for hb in range(2):
                nc.tensor.matmul(
                    pvl[hb][:], v_p[:, jj, hb, 0:DH + 1],
                    e_p[:, jj, hb, :],
                    start=(jt == 0), stop=(jt == JT - 1))

        def evict(pvl):
            for hb in range(2):
                nc.vector.tensor_copy(den_st[hb * 64:hb * 64 + 1, :],
                                      pvl[hb][DH:DH + 1, :])
            sbs = []
            for hb in range(2):
                t = npool.tile([64, 512], BF, tag="nb")
                nc.vector.tensor_copy(t[:], pvl[hb][0:DH, :])
                sbs.append(t)
            return sbs

        for dt in range(8):
            kt_t = kts.pop(dt)
            pv_cur = [psP.tile([DH + 1, 512], F32, tag="pp",
                               name=f"pv{dt}_{hb}") for hb in range(2)]
            for half in range(2):
                v_t = vp.tile([128, 8, 2, DH + 1], VDT, tag="v")
                vgh = vg1 if dt < 4 else vg2
                dl = dt % 4
                for ci in range(2):
                    c = 2 * half + ci
                    nc.sync.dma_start(
                        out=v_t[:, ci * 4:(ci + 1) * 4, :, :]
                        .rearrange("p t h e -> p t (h e)"),
                        in_=vgh[c, :, :]
                        .rearrange("(t p) f -> p t f", p=128)
                        [:, :, 2 * dl * (DH + 1):(2 * dl + 2) * (DH + 1)])
                expt = expp.tile([128, 8, 2, 512], PVDT, tag="e")
                for jj in range(8):
                    jt = half * 8 + jj
                    ps = psS.tile([128, 2, 512], F32, tag="ss")
                    for hb in range(2):
                        nc.tensor.matmul(
                            ps[:, hb, :],
                            kt_t[hb * 64:hb * 64 + 64, jt // 4,
                                 (jt % 4) * 128:(jt % 4) * 128 + 128],
                            qt_all[hb * 64:hb * 64 + 64, dt, :],
                            start=True, stop=True)
                    if masked:
                        nc.scalar.activation(
                            expt[:, jj, :, :], ps[:], Exp,
                            bias=bias_sb[:, jt:jt + 1], scale=1.0 / 8.0)
                    else:
                        nc.scalar.activation(
                            expt[:, jj, :, :], ps[:], Exp,
                            bias=shift_sb[:], scale=1.0 / 8.0)
                    # previous half's P*V rides along, one pair per jj
                    if prevpv is not None:
                        emit_pv(prevpv, jj)
                        if jj == 7:
                            if prevpv[4] == 8:
                                pending = (prevpv[0], evict(prevpv[1]))
                            prevpv = None
                    if jj == 4 and half == 1 and pending is not None:
                        normalize(pending)
                        pending = None
                # one-time PE fillers: local K^T for d'-tile 1 in two
                # parts, and the remaining Q^T projection quarters
                if dt == 0:
                    local_k(1, kts[1], cps=(half,))
                    q_proj(1 + half)
                if dt == 1 and half == 0:
                    q_proj(3)
                prevpv = (dt, pv_cur, v_t, expt, half * 8)
            # prefetch next gathered K^T tile
            if dt + 1 < 8 and dt + 1 >= NLOC and dt + 1 not in kts:
                kts[dt + 1] = ktp.tile([128, 4, Q], BF, tag="kt",
                                       name=f"kt{dt + 1}")
                fetch_k(dt + 1, kts[dt + 1])
        # flush the last half's P*V, then its normalize
        for jj in range(8):
            emit_pv(prevpv, jj)
        if pending is not None:
            normalize(pending)
        normalize((7, evict(prevpv[1])))

        # ---- output projection + residual + LayerNorm ----
        for it in range(IT):
            xq_t = xqp.tile([128, 1024], F32, tag="xq")
            nc.sync.dma_start(out=xq_t[:], in_=xq_ap[:, it, :])
            ps_r = psS.tile([128, 2, 512], F32, tag="ss")
            for mh in range(2):
                for kt in range(8):
                    nc.tensor.matmul(
                        ps_r[:, mh, :],
                        probt[:, kt, it * 128:(it + 1) * 128],
                        wo_sb[:, kt, mh * 512:(mh + 1) * 512],
                        start=(kt == 0), stop=(kt == 7))
            h_sb = lnp.tile([128, 1024], F32, tag="ln")
            nc.vector.tensor_add(h_sb[:], ps_r.rearrange("p a b -> p (a b)"),
                                 xq_t[:])
            stats = statp.tile([128, 2, 6], F32)
            nc.vector.bn_stats(stats[:, 0, :], h_sb[:, 0:512])
            nc.vector.bn_stats(stats[:, 1, :], h_sb[:, 512:1024])
            mv = statp.tile([128, 2], F32)
            nc.vector.bn_aggr(mv[:], stats[:])
            std = statp.tile([128, 1], F32)
            nc.scalar.activation(std[:], mv[:, 1:2], Sqrt,
                                 bias=eps_sb[:], scale=1.0)
            rstd = statp.tile([128, 1], F32)
            nc.vector.reciprocal(rstd[:], std[:])
            t1 = lnp.tile([128, 1024], F32, tag="ln")
            nc.vector.tensor_scalar(
                t1[:], h_sb[:], mv[:, 0:1], rstd[:],
                op0=mybir.AluOpType.subtract, op1=mybir.AluOpType.mult)
            if plain_ln:
                out_t = t1
            else:
                t2 = lnp.tile([128, 1024], F32, tag="ln")
                nc.vector.tensor_mul(t2[:], t1[:], gamma_sb[:])
                out_t = lnp.tile([128, 1024], F32, tag="ln")
                nc.vector.tensor_add(out_t[:], t2[:], beta_sb[:])
            nc.sync.dma_start(y_ap[it * 128:(it + 1) * 128, :], out_t[:])


def build_module(split=True, masked=False, plain_ln=False):
    nc = bass.Bass("TRN2", target_bir_lowering=False, debug=False,
                   num_devices=N_CORES)
    hh = {
        "xt": nc.dram_tensor("xt", [D, L], BF, kind="ExternalInput"),
        "xo": nc.dram_tensor("xo", [D, Q], BF, kind="ExternalInput"),
        "xq": nc.dram_tensor("xq", [Q, D], F32, kind="ExternalInput"),
        "wq": nc.dram_tensor("wq", [D, D], BF, kind="ExternalInput"),
        "wk": nc.dram_tensor("wk", [D, D], BF, kind="ExternalInput"),
        "wv": nc.dram_tensor("wv", [D, D], BF, kind="ExternalInput"),
        "wo": nc.dram_tensor("wo", [D, D], BF, kind="ExternalInput"),
        "bias": nc.dram_tensor("bias", [16, 128], F32, kind="ExternalInput"),
        "gamma": nc.dram_tensor("gamma", [D], BF, kind="ExternalInput"),
        "beta": nc.dram_tensor("beta", [D], BF, kind="ExternalInput"),
        "y": nc.dram_tensor("y", [Q, D], F32, kind="ExternalOutput"),
    }
    with tile.TileContext(nc) as tc:
        _emit(nc, tc, hh, masked, plain_ln)
    if split:
        _split_waits(nc, 1)
    return nc


_CACHE = {}


def get_module(masked=False, plain_ln=False):
    key = ("nc", masked, plain_ln)
    if key not in _CACHE:
        _CACHE[key] = build_module(masked=masked, plain_ln=plain_ln)
    return _CACHE[key]


def prep_inputs(x, mask, w_q, w_k, w_v, w_o, ln_gamma, ln_beta):
    x = np.asarray(x, dtype=np.float32)
    mask = np.asarray(mask)
    shared = {
        "wq": np.ascontiguousarray(
            np.asarray(w_q, np.float32).transpose(1, 0, 2).reshape(D, D)
        ).astype(BF16),
        "wk": np.ascontiguousarray(
            np.asarray(w_k, np.float32).transpose(1, 0, 2).reshape(D, D)
        ).astype(BF16),
        "wv": np.ascontiguousarray(
            np.asarray(w_v, np.float32).transpose(1, 0, 2).reshape(D, D)
        ).astype(BF16),
        "wo": np.asarray(w_o, np.float32).reshape(D, D).astype(BF16),
        "gamma": np.asarray(ln_gamma, np.float32).astype(BF16),
        "beta": np.asarray(ln_beta, np.float32).astype(BF16),
    }
    xts = [np.ascontiguousarray(x[b].T).astype(BF16) for b in range(B)]
    in_maps = []
    for c in range(N_CORES):
        b, q0 = c // 4, (c % 4) * Q
        m = {
            "xt": xts[b],
            "xo": np.ascontiguousarray(xts[b][:, q0:q0 + Q]),
            "xq": np.ascontiguousarray(x[b, q0:q0 + Q, :]),
            "bias": (np.where(mask[b], 0.0, -1e9) - EXP_SHIFT).astype(
                np.float32).reshape(16, 128),
        }
        m.update(shared)
        in_maps.append(m)
    masked = not bool(mask.all())
    plain_ln = bool(np.all(np.asarray(ln_gamma) == 1.0)
                    and np.all(np.asarray(ln_beta) == 0.0))
    return in_maps, (masked, plain_ln)


def assemble(results):
    out = np.empty((B, L, D), dtype=np.float32)
    for c in range(N_CORES):
        b, q0 = c // 4, (c % 4) * Q
        out[b, q0:q0 + Q, :] = results[c]["y"]
    return out


def run(in_maps, mode=(False, False), **kwargs):
    nc = get_module(*mode)
    return bass_utils.run_bass_kernel_spmd(
        nc, in_maps, core_ids=list(range(N_CORES)), **kwargs)


def kernel(x, mask, w_q, w_k, w_v, w_o, ln_gamma, ln_beta):
    in_maps, mode = prep_inputs(x, mask, w_q, w_k, w_v, w_o,
                                ln_gamma, ln_beta)
    res = run(in_maps, mode)
    return assemble(res.results)


# revision 32
# speedup vs baseline: 1.0992x; 1.0992x over previous
"""Trainium2 Bass kernel: fused multi-head attention block (projections +
softmax attention + output projection + residual + LayerNorm).

Sharding: 8 cores = 2 batches x 4 token-chunks of 512. Each core projects
K/V/Q for its OWN 512 tokens; V (fp8, with fused ones column) and the
K^T tail (d'-tiles NLOC..7) are AllGather-ed across the 4-core batch
group. K^T for d'-tiles 0..NLOC-1 is recomputed locally over the full
sequence so attention starts while the gathers are in flight. Each core
then runs attention for its 512 queries over all 2048 keys, the output
projection, residual add and LayerNorm.

P*V runs in fp8e4m3 with DoubleRow matmuls (2 key-tiles per pass); exp
is shifted by -7 so softmax numerators fit fp8 range (denominator
scales identically, so the quotient is unchanged).

Device-side layouts (per core):
  xt   [1024, 2048] bf16  x[b]^T, natural token order (local K path)
  xo   [1024, 512]  bf16  x[b]^T own-token columns (projections)
  xq   [512, 1024]  f32   own-token rows of x[b] (residual input)
  wq/wk/wv [1024, 1024] bf16  [c, h*64+d] (head-minor)
  wo   [1024, 1024] bf16  [(h*64+d), m]
  bias [16, 128]    f32   additive key bias per key tile/partition
                          (mask bias - EXP_SHIFT)
  gamma/beta [1024] bf16
Output: y [512, 1024] f32.
"""

import contextlib

import numpy as np
import ml_dtypes

import concourse.bass as bass
import concourse.tile as tile
from concourse import mybir
from concourse import bass_utils

BF16 = ml_dtypes.bfloat16
N_CORES = 8
B, L, D, H, DH = 2, 2048, 1024, 16, 64
Q = L // 4          # tokens owned per core
CT = D // 128       # contraction tiles over features
JT = L // 128       # key tiles
IT = Q // 128       # query tiles per core
LN_EPS = 1e-5
GROUPS = [[0, 1, 2, 3], [4, 5, 6, 7]]
NLOC = 2            # d'-tiles whose K^T is recomputed locally

EXP_SHIFT = 0.0

F32 = mybir.dt.float32
BF = mybir.dt.bfloat16
E8 = mybir.dt.float8e4
PVDT = BF           # softmax numerators (P) stay bf16
VDT = E8            # V is gathered/stored in fp8e4m3 (stationary operand)
FV = H * (DH + 1)   # V row width incl ones columns
FVH = 8 * (DH + 1)  # half of it (8 heads)


def _split_waits(nc, maxw=1):
    """This walrus build rejects instructions with more than one sync wait;
    split excess waits into preceding NOPs on the same engine."""
    ctr = 0
    for fn in nc.m.functions:
        for bb in fn.blocks:
            new_insts = []
            for inst in bb.instructions:
                si = inst.sync_info
                if si is not None and len(si.on_wait) > maxw:
                    waits = list(si.on_wait)
                    excess, keep = waits[:-maxw], waits[-maxw:]
                    for i in range(0, len(excess), maxw):
                        ctr += 1
                        new_insts.append(mybir.InstNoOp(
                            name=f"waitsplit_nop_{ctr}",
                            engine=inst.engine,
                            sync_info=mybir.SyncInfo(
                                on_wait=excess[i:i + maxw], on_update=[]),
                            text_hint="waitsplit",
                        ))
                    si.on_wait = keep
                new_insts.append(inst)
            bb.instructions = new_insts
    return ctr


def _emit(nc, tc, hh, masked, plain_ln):
    Exp = mybir.ActivationFunctionType.Exp
    Sqrt = mybir.ActivationFunctionType.Sqrt
    DR = mybir.MatmulPerfMode.DoubleRow

    xt_ap = hh["xt"].ap().rearrange("(t p) l -> p t l", p=128)   # [128,8,2048]
    xo_ap = hh["xo"].ap().rearrange("(t p) q -> p t q", p=128)   # [128,8,512]
    wq_ap = hh["wq"].ap().rearrange("(t p) d -> p t d", p=128)
    wk_ap = hh["wk"].ap().rearrange("(t p) d -> p t d", p=128)
    wv_ap = hh["wv"].ap().rearrange("(t p) d -> p t d", p=128)
    wo_ap = hh["wo"].ap().rearrange("(t p) d -> p t d", p=128)
    bias_ap = hh["bias"].ap().rearrange("a b -> b a")            # [128,16]
    xq_ap = hh["xq"].ap().rearrange("(t p) d -> p t d", p=128)   # [128,4,1024]
    y_ap = hh["y"].ap()

    def bcast_dram(h1d, parts=128):
        a = h1d.ap()
        return bass.AP(tensor=a.tensor, offset=a.offset,
                       ap=[[0, parts]] + list(a.ap))

    with contextlib.ExitStack() as ctx:
        dram = ctx.enter_context(tc.tile_pool(name="dram", bufs=1,
                                              space="DRAM"))
        const = ctx.enter_context(tc.tile_pool(name="const", bufs=1))
        xtp = ctx.enter_context(tc.tile_pool(name="xtp", bufs=4))
        wpool = ctx.enter_context(tc.tile_pool(name="wpool", bufs=2))
        wkp = ctx.enter_context(tc.tile_pool(name="wkp", bufs=1))
        vctp = ctx.enter_context(tc.tile_pool(name="vctp", bufs=1))
        qtp = ctx.enter_context(tc.tile_pool(name="qtp", bufs=1))
        ktp = ctx.enter_context(tc.tile_pool(name="ktp", bufs=4))
        vp = ctx.enter_context(tc.tile_pool(name="vp", bufs=2))
        expp = ctx.enter_context(tc.tile_pool(name="expp", bufs=2))
        ptp = ctx.enter_context(tc.tile_pool(name="ptp", bufs=1))
        npool = ctx.enter_context(tc.tile_pool(name="npool", bufs=2))
        xqp = ctx.enter_context(tc.tile_pool(name="xqp", bufs=2))
        lnp = ctx.enter_context(tc.tile_pool(name="lnp", bufs=2))
        statp = ctx.enter_context(tc.tile_pool(name="statp", bufs=4))
        psS = ctx.enter_context(tc.tile_pool(name="psS", bufs=2, space="PSUM"))
        psP = ctx.enter_context(tc.tile_pool(name="psP", bufs=4, space="PSUM"))

        # DRAM bounce buffers; gather order is V(heads 0-7), K tail,
        # V(heads 8-15) so each lands just before its first consumer
        vc1 = dram.tile([Q, FVH], VDT)
        vg1 = dram.tile([4, Q, FVH], VDT)
        vc2 = dram.tile([Q, FVH], VDT)
        vg2 = dram.tile([4, Q, FVH], VDT)
        KD = 8 - NLOC
        kc = dram.tile([KD * 128, Q], BF)
        kg = dram.tile([4, KD * 128, Q], BF)

        # ---- constants / small loads ----
        eps_sb = const.tile([128, 1], F32)
        nc.vector.memset(eps_sb[:], LN_EPS)
        shift_sb = const.tile([128, 1], F32)
        nc.vector.memset(shift_sb[:], -EXP_SHIFT)
        ones64 = const.tile([65, 64], F32)
        nc.vector.memset(ones64[:], 1.0)
        bias_sb = const.tile([128, 16], F32)
        den_st = const.tile([65, 512], F32)
        nc.vector.memset(den_st[:], 1.0)
        rdiv_st = const.tile([65, 512], F32)
        gamma_sb = const.tile([128, 1024], BF)
        beta_sb = const.tile([128, 1024], BF)

        # ---- input streams: xo/xt on SP queue, weights on ACT queue ----
        xo_sb = const.tile([128, CT, Q], BF)
        nc.sync.dma_start(out=xo_sb[:], in_=xo_ap)
        wk_sb = wkp.tile([128, CT, 1024], BF)
        nc.sync.dma_start(out=wk_sb[:], in_=wk_ap)
        xt_pre = []
        for c in range(4):
            t = xtp.tile([128, CT, 512], BF, tag="xt", name=f"xtpre{c}")
            nc.gpsimd.dma_start(out=t[:],
                                in_=xt_ap[:, :, c * 512:(c + 1) * 512])
            xt_pre.append(t)
        warm = const.tile([1, 1], F32)
        nc.scalar.activation(warm[:], eps_sb[0:1, 0:1], Exp,
                             bias=0.0, scale=1.0)
        wv_sb = wpool.tile([128, CT, 1024], BF, tag="w")
        nc.scalar.dma_start(out=wv_sb[:, 0:4, :], in_=wv_ap[:, 0:4, :])
        nc.scalar.dma_start(out=wv_sb[:, 4:8, :], in_=wv_ap[:, 4:8, :])
        wq_sb = wpool.tile([128, CT, 1024], BF, tag="w")
        nc.scalar.dma_start(out=wq_sb[:], in_=wq_ap)

        # ---- V projection (own tokens): [token 128][h*64] + ones col ----
        vct = vctp.tile([128, 4, H, DH + 1], VDT)
        nc.vector.memset(vct[:, :, :, DH:DH + 1], 1.0)
        for lt in range(4):
            ps = psS.tile([128, 2, 512], F32, tag="ss")
            for nt in range(2):
                for ct in range(CT):
                    nc.tensor.matmul(
                        ps[:, nt, :],
                        xo_sb[:, ct, lt * 128:(lt + 1) * 128],
                        wv_sb[:, ct, nt * 512:(nt + 1) * 512],
                        start=(ct == 0), stop=(ct == CT - 1))
            nc.vector.tensor_copy(
                vct[:, lt, :, 0:DH],
                ps.rearrange("p n (h d) -> p (n h) d", h=8))
        nc.gpsimd.dma_start(
            out=vc1[:].rearrange("(t p) f -> p t f", p=128),
            in_=vct[:, :, 0:8, :].rearrange("p t h e -> p t (h e)"))
        nc.gpsimd.collective_compute(
            "AllGather", mybir.AluOpType.bypass, replica_groups=GROUPS,
            ins=[vc1[:].opt()], outs=[vg1[:].opt()])

        # ---- Q^T projection part 1 (d'-tiles 0/1) so attention can
        # start; local K^T for d'-tile 0 runs before the own-key pass ----
        probt = ptp.tile([128, 8, Q], BF)
        kct = probt
        qt_all = qtp.tile([128, 8, Q], BF)

        def q_proj(dtp):
            ps = psS.tile([128, 2, 512], F32, tag="ss")
            for half in range(2):
                dt = 2 * dtp + half
                for ct in range(CT):
                    nc.tensor.matmul(
                        ps[:, half, :],
                        wq_sb[:, ct, dt * 128:(dt + 1) * 128],
                        xo_sb[:, ct, :],
                        start=(ct == 0), stop=(ct == CT - 1))
            nc.vector.tensor_copy(qt_all[:, 2 * dtp:2 * dtp + 2, :], ps[:])

        # ---- K^T projection (own keys, d'-tiles NLOC..7 only) ----
        # staged in the probt tile (dead until phase B; kc DMA orders reuse)
        def k_own():
            for i, dt in enumerate(range(NLOC, 8)):
                half = i % 2
                if half == 0:
                    ps = psS.tile([128, 2, 512], F32, tag="ss")
                for ct in range(CT):
                    nc.tensor.matmul(
                        ps[:, half, :],
                        wk_sb[:, ct, dt * 128:(dt + 1) * 128],
                        xo_sb[:, ct, :],
                        start=(ct == 0), stop=(ct == CT - 1))
                if half == 1 or i == KD - 1:
                    nc.vector.tensor_copy(
                        kct[:, i - half:i + 1, :], ps[:, 0:half + 1, :])
            nc.gpsimd.dma_start(
                out=kc[:].rearrange("(t p) q -> p t q", p=128),
                in_=kct[:, 0:KD, :])
            nc.gpsimd.collective_compute(
                "AllGather", mybir.AluOpType.bypass, replica_groups=GROUPS,
                ins=[kc[:].opt()], outs=[kg[:].opt()])
            nc.gpsimd.dma_start(
                out=vc2[:].rearrange("(t p) f -> p t f", p=128),
                in_=vct[:, :, 8:16, :].rearrange("p t h e -> p t (h e)"))
            nc.gpsimd.collective_compute(
                "AllGather", mybir.AluOpType.bypass, replica_groups=GROUPS,
                ins=[vc2[:].opt()], outs=[vg2[:].opt()])

        # remaining loads for phases B/C
        wo_sb = wpool.tile([128, CT, 1024], BF, tag="w")
        nc.scalar.dma_start(out=wo_sb[:], in_=wo_ap)
        nc.scalar.dma_start(out=bias_sb[:], in_=bias_ap)
        nc.scalar.dma_start(out=gamma_sb[:], in_=bcast_dram(hh["gamma"]))
        nc.scalar.dma_start(out=beta_sb[:], in_=bcast_dram(hh["beta"]))

        # ---- attention loop over d'-tiles (= head pairs) ----
        def local_k(dt, kt_t, pre=None, cps=(0, 1)):
            # local K^T over the full (natural-order) sequence,
            # x^T streamed chunk-by-chunk from DRAM
            for cp in cps:
                ps = psS.tile([128, 2, 512], F32, tag="ss")
                for hf in range(2):
                    c = 2 * cp + hf
                    if pre is not None and c < len(pre):
                        xt_c = pre[c]
                    else:
                        xt_c = xtp.tile([128, CT, 512], BF, tag="xt")
                        nc.sync.dma_start(
                            out=xt_c[:],
                            in_=xt_ap[:, :, c * 512:(c + 1) * 512])
                    for ct in range(CT):
                        nc.tensor.matmul(
                            ps[:, hf, :],
                            wk_sb[:, ct, dt * 128:(dt + 1) * 128],
                            xt_c[:, ct, :],
                            start=(ct == 0), stop=(ct == CT - 1))
                nc.vector.tensor_copy(
                    kt_t[:, 2 * cp:2 * cp + 2, :], ps[:])

        def fetch_k(dt, kt_t):
            nc.sync.dma_start(
                out=kt_t[:],
                in_=kg[:, (dt - NLOC) * 128:(dt - NLOC + 1) * 128, :]
                .rearrange("c p q -> p c q"))

        def normalize(pend):
            pdt, pv_sbs = pend
            nc.vector.reciprocal(rdiv_st[:], den_st[:])
            for hb in range(2):
                ps_b = psP.tile([64, 512], F32, tag="pp", name=f"bb{hb}")
                nc.tensor.matmul(ps_b[:],
                                 ones64[hb * 64:hb * 64 + 1, :],
                                 rdiv_st[hb * 64:hb * 64 + 1, :],
                                 start=True, stop=True)
                nc.vector.tensor_mul(
                    probt[hb * 64:hb * 64 + 64, pdt, :],
                    pv_sbs[hb][:], ps_b[:])

        kts = {0: ktp.tile([128, 4, Q], BF, tag="kt", name="kt0"),
               1: ktp.tile([128, 4, Q], BF, tag="kt", name="kt1")}
        k_own()
        local_k(0, kts[0], pre=xt_pre)
        q_proj(0)
        pending = None          # (pdt, pv_sbs) awaiting normalize
        prevpv = None           # (pdt, pv_list, v_t, expt, base_jt)

        def emit_pv(pp, jj):
            pdt, pvl, v_p, e_p, bjt = pp
            jt = bjt + jj
            for hb in range(2):
                nc.tensor.matmul(
                    pvl[hb][:], v_p[:, jj, hb, 0:DH + 1],
                    e_p[:, jj, hb, :],
                    start=(jt == 0), stop=(jt == JT - 1))

        def evict(pvl):
            for hb in range(2):
                nc.vector.tensor_copy(den_st[hb * 64:hb * 64 + 1, :],
                                      pvl[hb][DH:DH + 1, :])
            sbs = []
            for hb in range(2):
                t = npool.tile([64, 512], BF, tag="nb")
                nc.vector.tensor_copy(t[:], pvl[hb][0:DH, :])
                sbs.append(t)
            return sbs

        for dt in range(8):
            kt_t = kts.pop(dt)
            pv_cur = [psP.tile([DH + 1, 512], F32, tag="pp",
                               name=f"pv{dt}_{hb}") for hb in range(2)]
            for half in range(2):
                v_t = vp.tile([128, 8, 2, DH + 1], VDT, tag="v")
                vgh = vg1 if dt < 4 else vg2
                dl = dt % 4
                for ci in range(2):
                    c = 2 * half + ci
                    nc.sync.dma_start(
                        out=v_t[:, ci * 4:(ci + 1) * 4, :, :]
                        .rearrange("p t h e -> p t (h e)"),
                        in_=vgh[c, :, :]
                        .rearrange("(t p) f -> p t f", p=128)
                        [:, :, 2 * dl * (DH + 1):(2 * dl + 2) * (DH + 1)])
                expt = expp.tile([128, 8, 2, 512], PVDT, tag="e")
                for jj in range(8):
                    jt = half * 8 + jj
                    ps = psS.tile([128, 2, 512], F32, tag="ss")
                    for hb in range(2):
                        nc.tensor.matmul(
                            ps[:, hb, :],
                            kt_t[hb * 64:hb * 64 + 64, jt // 4,
                                 (jt % 4) * 128:(jt % 4) * 128 + 128],
                            qt_all[hb * 64:hb * 64 + 64, dt, :],
                            start=True, stop=True)
                    if masked:
                        nc.scalar.activation(
                            expt[:, jj, :, :], ps[:], Exp,
                            bias=bias_sb[:, jt:jt + 1], scale=1.0 / 8.0)
                    else:
                        nc.scalar.activation(
                            expt[:, jj, :, :], ps[:], Exp,
                            bias=shift_sb[:], scale=1.0 / 8.0)
                    # previous half's P*V rides along, one pair per jj
                    if prevpv is not None:
                        emit_pv(prevpv, jj)
                        if jj == 7:
                            if prevpv[4] == 8:
                                pending = (prevpv[0], evict(prevpv[1]))
                            prevpv = None
                    if jj == 4 and half == 1 and pending is not None:
                        normalize(pending)
                        pending = None
                # one-time PE fillers: local K^T for d'-tile 1 in two
                # parts, and the remaining Q^T projection quarters
                if dt == 0:
                    local_k(1, kts[1], cps=(half,))
                    q_proj(1 + half)
                if dt == 1 and half == 0:
                    q_proj(3)
                prevpv = (dt, pv_cur, v_t, expt, half * 8)
            # prefetch next gathered K^T tile
            if dt + 1 < 8 and dt + 1 >= NLOC and dt + 1 not in kts:
                kts[dt + 1] = ktp.tile([128, 4, Q], BF, tag="kt",
                                       name=f"kt{dt + 1}")
                fetch_k(dt + 1, kts[dt + 1])
        # flush the last half's P*V, then its normalize
        for jj in range(8):
            emit_pv(prevpv, jj)
        if pending is not None:
            normalize(pending)
        normalize((7, evict(prevpv[1])))

        # ---- output projection + residual + LayerNorm ----
        for it in range(IT):
            xq_t = xqp.tile([128, 1024], F32, tag="xq")
            nc.sync.dma_start(out=xq_t[:], in_=xq_ap[:, it, :])
            ps_r = psS.tile([128, 2, 512], F32, tag="ss")
            for mh in range(2):
                for kt in range(8):
                    nc.tensor.matmul(
                        ps_r[:, mh, :],
                        probt[:, kt, it * 128:(it + 1) * 128],
                        wo_sb[:, kt, mh * 512:(mh + 1) * 512],
                        start=(kt == 0), stop=(kt == 7))
            h_sb = lnp.tile([128, 1024], F32, tag="ln")
            nc.vector.tensor_add(h_sb[:], ps_r.rearrange("p a b -> p (a b)"),
                                 xq_t[:])
            stats = statp.tile([128, 2, 6], F32)
            nc.vector.bn_stats(stats[:, 0, :], h_sb[:, 0:512])
            nc.vector.bn_stats(stats[:, 1, :], h_sb[:, 512:1024])
            mv = statp.tile([128, 2], F32)
            nc.vector.bn_aggr(mv[:], stats[:])
            std = statp.tile([128, 1], F32)
            nc.scalar.activation(std[:], mv[:, 1:2], Sqrt,
                                 bias=eps_sb[:], scale=1.0)
            rstd = statp.tile([128, 1], F32)
            nc.vector.reciprocal(rstd[:], std[:])
            t1 = lnp.tile([128, 1024], F32, tag="ln")
            nc.vector.tensor_scalar(
                t1[:], h_sb[:], mv[:, 0:1], rstd[:],
                op0=mybir.AluOpType.subtract, op1=mybir.AluOpType.mult)
            if plain_ln:
                out_t = t1
            else:
                t2 = lnp.tile([128, 1024], F32, tag="ln")
                nc.vector.tensor_mul(t2[:], t1[:], gamma_sb[:])
                out_t = lnp.tile([128, 1024], F32, tag="ln")
                nc.vector.tensor_add(out_t[:], t2[:], beta_sb[:])
            nc.sync.dma_start(y_ap[it * 128:(it + 1) * 128, :], out_t[:])


def build_module(split=True, masked=False, plain_ln=False):
    nc = bass.Bass("TRN2", target_bir_lowering=False, debug=False,
                   num_devices=N_CORES)
    hh = {
        "xt": nc.dram_tensor("xt", [D, L], BF, kind="ExternalInput"),
        "xo": nc.dram_tensor("xo", [D, Q], BF, kind="ExternalInput"),
        "xq": nc.dram_tensor("xq", [Q, D], F32, kind="ExternalInput"),
        "wq": nc.dram_tensor("wq", [D, D], BF, kind="ExternalInput"),
        "wk": nc.dram_tensor("wk", [D, D], BF, kind="ExternalInput"),
        "wv": nc.dram_tensor("wv", [D, D], BF, kind="ExternalInput"),
        "wo": nc.dram_tensor("wo", [D, D], BF, kind="ExternalInput"),
        "bias": nc.dram_tensor("bias", [16, 128], F32, kind="ExternalInput"),
        "gamma": nc.dram_tensor("gamma", [D], BF, kind="ExternalInput"),
        "beta": nc.dram_tensor("beta", [D], BF, kind="ExternalInput"),
        "y": nc.dram_tensor("y", [Q, D], F32, kind="ExternalOutput"),
    }
    with tile.TileContext(nc) as tc:
        _emit(nc, tc, hh, masked, plain_ln)
    if split:
        _split_waits(nc, 1)
    return nc


_CACHE = {}


def get_module(masked=False, plain_ln=False):
    key = ("nc", masked, plain_ln)
    if key not in _CACHE:
        _CACHE[key] = build_module(masked=masked, plain_ln=plain_ln)
    return _CACHE[key]


def prep_inputs(x, mask, w_q, w_k, w_v, w_o, ln_gamma, ln_beta):
    x = np.asarray(x, dtype=np.float32)
    mask = np.asarray(mask)
    shared = {
        "wq": np.ascontiguousarray(
            np.asarray(w_q, np.float32).transpose(1, 0, 2).reshape(D, D)
        ).astype(BF16),
        "wk": np.ascontiguousarray(
            np.asarray(w_k, np.float32).transpose(1, 0, 2).reshape(D, D)
        ).astype(BF16),
        "wv": np.ascontiguousarray(
            np.asarray(w_v, np.float32).transpose(1, 0, 2).reshape(D, D)
        ).astype(BF16),
        "wo": np.asarray(w_o, np.float32).reshape(D, D).astype(BF16),
        "gamma": np.asarray(ln_gamma, np.float32).astype(BF16),
        "beta": np.asarray(ln_beta, np.float32).astype(BF16),
    }
    xts = [np.ascontiguousarray(x[b].T).astype(BF16) for b in range(B)]
    in_maps = []
    for c in range(N_CORES):
        b, q0 = c // 4, (c % 4) * Q
        m = {
            "xt": xts[b],
            "xo": np.ascontiguousarray(xts[b][:, q0:q0 + Q]),
            "xq": np.ascontiguousarray(x[b, q0:q0 + Q, :]),
            "bias": (np.where(mask[b], 0.0, -1e9) - EXP_SHIFT).astype(
                np.float32).reshape(16, 128),
        }
        m.update(shared)
        in_maps.append(m)
    masked = not bool(mask.all())
    plain_ln = bool(np.all(np.asarray(ln_gamma) == 1.0)
                    and np.all(np.asarray(ln_beta) == 0.0))
    return in_maps, (masked, plain_ln)


def assemble(results):
    out = np.empty((B, L, D), dtype=np.float32)
    for c in range(N_CORES):
        b, q0 = c // 4, (c % 4) * Q
        out[b, q0:q0 + Q, :] = results[c]["y"]
    return out


def run(in_maps, mode=(False, False), **kwargs):
    nc = get_module(*mode)
    return bass_utils.run_bass_kernel_spmd(
        nc, in_maps, core_ids=list(range(N_CORES)), **kwargs)


def kernel(x, mask, w_q, w_k, w_v, w_o, ln_gamma, ln_beta):
    in_maps, mode = prep_inputs(x, mask, w_q, w_k, w_v, w_o,
                                ln_gamma, ln_beta)
    res = run(in_maps, mode)
    return assemble(res.results)
